# revision 23
# baseline (speedup 1.0000x reference)
"""APS (adaptive polyphase sampling) layer kernel for Trainium2, 8 NeuronCores.

Problem: inputs [32,128,128,128] f32. Split into 4 polyphase components
comps[k] = inputs[:, i::2, j::2, :] (k = i*2+j). Compute L2 norm of each
component over all elements, pick argmax k*, output (comps[k*], k*).

Strategy (data-parallel over batch, 8 cores x 4 batches):
  - Each core streams its 32MB shard as 8 contiguous-run tiles of 4MB
    ([128 partitions = 2 batches x 64 rows of one h-parity] x [32KB free]).
  - ScalarE squares each tile with accum_out -> per-(h-parity, w-parity)
    partial sums; VectorE reduces; AllReduce over the 8 cores; argmax and
    the (i,j) offsets are computed on-device.
  - The winning component is copied with one dynamic-offset DRAM->DRAM DMA
    per core (strided read, contiguous write).
"""

import numpy as np

import concourse.bass as bass
from concourse import mybir

N_CORES = 8
B, H, W, C = 32, 128, 128, 128
BS = B // N_CORES          # batches per core
HH, WW = H // 2, W // 2
NT = 8                     # tiles per core: (ii=2) x (b2=2) x (wq=2)
NBUF = 4                   # tile ring depth (hint tiles stay resident)
TILE_F = 64 * C            # free elems per tile partition-line (wl*c) = 8192
HALF_F = TILE_F // 2       # per parity-j slice = 4096

# Speculative-output hint: component whose data is written to `out` during
# phase A (overlapped with the input streaming). If the argmax turns out
# different, a conditional rewrite fixes `out` (correct for any input,
# fastest when the hint hits).
K_HINT = 3
I_HINT, J_HINT = K_HINT // 2, K_HINT % 2

F32 = mybir.dt.float32
I32 = mybir.dt.int32

def _build(variant: str = "full"):
    # variant "full": collective allreduce + branchy gather (correct kernel)
    # variant "sim":  no collective, static gather (TimelineSim-compatible)
    nc = bass.Bass("TRN2", num_devices=N_CORES)

    x = nc.dram_tensor("x", [BS, H, W, C], F32, kind="ExternalInput")
    cvec_d = nc.dram_tensor("consts", [1, 16], F32, kind="ExternalInput")
    out = nc.dram_tensor("out", [BS, HH, WW, C], F32, kind="ExternalOutput")
    idx_out = nc.dram_tensor("idx", [1, 1], I32, kind="ExternalOutput")

    cc_in = nc.dram_tensor("cc_in", [1, 4], F32)
    cc_out = nc.dram_tensor("cc_out", [1, 4], F32, addr_space="Shared")

    # tile (ii, b2, wq): partitions=(bl,hh), free=(wl,c); all strides
    # merge into [128 x 32KB-contiguous] DMA lines.
    x_t = x[:].rearrange(
        "(b2 bl) (hh ii) (wq wl) c -> ii b2 wq (bl hh) (wl c)",
        b2=2, ii=2, wq=2,
    )
    # final gather view: [ii, jj, b, hh, ww, c]
    x_g = x[:].rearrange("b (hh ii) (ww jj) c -> ii jj b hh ww c", ii=2, jj=2)
    # speculative-write destination view: [b2, wq, (bl hh), (wl c)]
    out_sp = out[:].rearrange(
        "(b2 bl) hh (wq wl) c -> b2 wq (bl hh) (wl c)", b2=2, wq=2,
    )
    # load order: hint-parity tiles LAST — with NBUF=4 they stay resident in
    # the ring, and their speculative output writes are queued after all
    # loads, draining while the compute tail runs.
    tiles = [(1 - I_HINT, b2, wq) for b2 in range(2) for wq in range(2)]
    tiles += [(I_HINT, b2, wq) for b2 in range(2) for wq in range(2)]

    from contextlib import ExitStack

    with ExitStack() as ctx:
        e = ctx.enter_context
        tbuf = e(nc.sbuf_tensor([128, NBUF * TILE_F], F32))
        scratch = e(nc.sbuf_tensor([128, TILE_F], F32))
        acc = e(nc.sbuf_tensor([128, 16], F32))
        redT = e(nc.sbuf_tensor([1, 512], F32))
        red4 = e(nc.sbuf_tensor([128, 4], F32))
        norms4 = e(nc.sbuf_tensor([1, 4], F32))
        normsG = e(nc.sbuf_tensor([1, 4], F32))
        onehot = e(nc.sbuf_tensor([1, 4], F32))
        junk = e(nc.sbuf_tensor([1, 4], F32))
        maxv = e(nc.sbuf_tensor([1, 1], F32))
        kf = e(nc.sbuf_tensor([1, 1], F32))
        ki = e(nc.sbuf_tensor([1, 1], I32))
        cvec = e(nc.sbuf_tensor([1, 16], F32))
        s_load = e(nc.semaphore("s_load"))
        s_const = e(nc.semaphore("s_const"))
        s_act = e(nc.semaphore("s_act"))
        s_dve = e(nc.semaphore("s_dve"))
        s_flat = e(nc.semaphore("s_flat"))
        s_ccin = e(nc.semaphore("s_ccin"))
        s_cc = e(nc.semaphore("s_cc"))
        s_ccout = e(nc.semaphore("s_ccout"))
        s_idx = e(nc.semaphore("s_idx"))
        s_out = e(nc.semaphore("s_out"))
        s_spec = e(nc.semaphore("s_spec"))
        s_chain = e(nc.semaphore("s_chain"))
        block = e(nc.Block())
        @block.gpsimd
        def _(gp):
            gp.dma_start(out=cvec[:], in_=cvec_d[:]).then_inc(s_const, 16)
            spec = variant != "sim_nogather"
            for t in range(NT):
                ii, b2, wq = tiles[t]
                if t >= NBUF:
                    gp.wait_ge(s_act, t - NBUF + 1)
                slot = t % NBUF
                gp.dma_start(
                    out=tbuf[:, slot * TILE_F:(slot + 1) * TILE_F],
                    in_=x_t[ii, b2, wq],
                ).then_inc(s_load, 16)
            if spec:
                # speculative output writes: J_HINT-parity w-columns of the
                # (still-resident) hint tiles -> out (strided SBUF read,
                # contiguous DRAM write). Queued after all loads on the same
                # SWDGE queue: each SDMA engine drains its ring in order, so
                # these reads see the loads' writes, and they drain while the
                # compute tail runs.
                for t in range(NT):
                    ii, b2, wq = tiles[t]
                    if ii != I_HINT:
                        continue
                    slot = t % NBUF
                    sbv = tbuf[
                        :, slot * TILE_F:(slot + 1) * TILE_F
                    ].rearrange("p (wl2 j c) -> p j wl2 c", j=2, c=C)
                    gp.dma_start(
                        out=out_sp[b2, wq], in_=sbv[:, J_HINT],
                    ).then_inc(s_spec, 16)

            # cross-partition flatten of per-partition accumulators
            gp.wait_ge(s_dve, 1)
            gp.dma_start(out=redT[:], in_=red4[:]).then_inc(s_flat, 16)

            # AllReduce the 4 partial sums across cores
            if variant == "full":
                gp.wait_ge(s_dve, 2)
                gp.dma_start(out=cc_in[:], in_=norms4[:]).then_inc(s_ccin, 16)
                gp.wait_ge(s_ccin, 16)
                gp.collective_compute(
                    "AllReduce",
                    mybir.AluOpType.add,
                    replica_groups=[list(range(N_CORES))],
                    ins=[cc_in[:]],
                    outs=[cc_out[:]],
                ).then_inc(s_cc)
                gp.wait_ge(s_cc, 1)
                gp.dma_start(out=normsG[:], in_=cc_out[:]).then_inc(s_ccout, 16)

            # index output + conditional rewrite when the hint missed
            gp.wait_ge(s_dve, 3)
            gp.dma_start(out=idx_out[:], in_=ki[:]).then_inc(s_idx, 16)
            if variant == "full":
                kv = nc.values_load(
                    ki[0:1, 0:1], engines=(mybir.EngineType.Pool,),
                    min_val=0, max_val=3, skip_runtime_bounds_check=True,
                )
                for kk in range(4):
                    if kk == K_HINT:
                        continue  # hint hit: speculative writes already did it
                    cond = nc.snap(
                        kv, engines=bass.OrderedSet([mybir.EngineType.Pool])
                    ) == kk
                    with gp.If(cond):
                        # all speculative writes must have landed before the
                        # rewrite touches the same addresses
                        gp.wait_ge(s_spec, 16 * 4)
                        for b in range(BS):
                            gp.dma_start(
                                out=out[b], in_=x_g[kk // 2, kk % 2, b],
                            ).then_inc(s_out, 16)
                        gp.wait_ge(s_out, 16 * BS)
            gp.wait_ge(s_idx, 16)
            if variant != "sim_nogather":
                gp.wait_ge(s_spec, 16 * 4)

        @block.scalar
        def _(act):
            for t in range(NT):
                ii, b2, wq = tiles[t]
                act.wait_ge(s_load, 16 * (t + 1))
                slot = t % NBUF
                sb = tbuf[:, slot * TILE_F:(slot + 1) * TILE_F].rearrange(
                    "p (wl2 j c) -> p j wl2 c", j=2, c=C,
                )
                for j in range(2):
                    s = j * 8 + ii * 4 + b2 * 2 + wq
                    ins = act.activation(
                        out=scratch[:, j * HALF_F:(j + 1) * HALF_F].rearrange(
                            "p (a c) -> p a c", c=C,
                        ),
                        in_=sb[:, j],
                        func=mybir.ActivationFunctionType.Square,
                        accum_out=acc[:, s:s + 1],
                    )
                ins.then_inc(s_act, 1)

        @block.vector
        def _(dve):
            # acc slots: s = j*8 + ii*4 + r  ->  [p, (j ii r)] -> sum r
            dve.wait_ge(s_act, NT)
            dve.tensor_reduce(
                out=red4[:],
                in_=acc[:].rearrange("p (j ii r) -> p ii j r", j=2, ii=2),
                axis=mybir.AxisListType.X,
                op=mybir.AluOpType.add,
            ).then_inc(s_dve, 1)

            # redT[0, p*4 + k] -> norms4[0, k]
            dve.wait_ge(s_flat, 16)
            dve.tensor_reduce(
                out=norms4[:],
                in_=redT[:].rearrange("q (p k) -> q k p", k=4),
                axis=mybir.AxisListType.X,
                op=mybir.AluOpType.add,
            ).then_inc(s_dve, 1)

            # argmax arithmetic on the globally-reduced norms
            # NB: consecutive dependent ops on the same engine need explicit
            # waits (deep pipeline; no same-engine RAW interlock).
            cb = 0
            if variant == "full":
                dve.wait_ge(s_ccout, 16)
            else:
                dve.wait_ge(s_dve, 2)
                dve.tensor_copy(out=normsG[:], in_=norms4[:]).then_inc(s_chain, 1)
                dve.wait_ge(s_chain, 1)
                cb = 1
            dve.wait_ge(s_const, 16)
            dve.tensor_reduce(
                out=maxv[:], in_=normsG[:],
                axis=mybir.AxisListType.X, op=mybir.AluOpType.max,
            ).then_inc(s_chain, 1)
            dve.wait_ge(s_chain, cb + 1)
            dve.tensor_scalar(
                out=onehot[:], in0=normsG[:],
                scalar1=maxv[0:1, 0:1], scalar2=None,
                op0=mybir.AluOpType.is_ge,
            ).then_inc(s_chain, 1)
            dve.wait_ge(s_chain, cb + 2)
            dve.tensor_tensor(
                out=junk[:], in0=onehot[:], in1=cvec[0:1, 0:4],
                op=mybir.AluOpType.mult,
            ).then_inc(s_chain, 1)
            dve.wait_ge(s_chain, cb + 3)
            dve.tensor_reduce(
                out=kf[:], in_=junk[:],
                axis=mybir.AxisListType.X, op=mybir.AluOpType.add,
            ).then_inc(s_chain, 1)
            dve.wait_ge(s_chain, cb + 4)
            dve.tensor_copy(out=ki[:], in_=kf[:]).then_inc(s_dve, 1)

    return nc


def _consts() -> np.ndarray:
    c = np.zeros((1, 16), dtype=np.float32)
    c[0, 0:4] = [0, 1, 2, 3]    # k
    c[0, 4:8] = [0, 0, 1, 1]    # i = k // 2
    c[0, 8:12] = [0, 1, 0, 1]   # j = k % 2
    return c


_RUNNER_CACHE = None


def _build_runner():
    """Compile the bass module once and return a reusable jitted executor
    (mirrors bass2jax.run_bass_via_pjrt's multi-core path, minus donation,
    so repeat kernel() calls skip re-lowering)."""
    import jax
    from jax.experimental.shard_map import shard_map
    from jax.sharding import Mesh, NamedSharding, PartitionSpec

    from concourse import bass2jax

    nc = _build("full")
    bass2jax.install_neuronx_cc_hook()
    partition_name = nc.partition_id_tensor.name if nc.partition_id_tensor else None

    in_names, out_names, out_avals, zero_outs = [], [], [], []
    for alloc in nc.m.functions[0].allocations:
        if not isinstance(alloc, mybir.MemoryLocationSet):
            continue
        name = alloc.memorylocations[0].name
        if alloc.kind == "ExternalInput":
            if name != partition_name:
                in_names.append(name)
        elif alloc.kind == "ExternalOutput":
            shape = tuple(alloc.tensor_shape)
            dtype = mybir.dt.np(alloc.dtype)
            out_avals.append(jax.core.ShapedArray(shape, dtype))
            out_names.append(name)
            zero_outs.append(np.zeros(shape, dtype))

    n_params = len(in_names)
    all_in_names = list(in_names) + list(out_names)
    if partition_name is not None:
        all_in_names.append(partition_name)

    def _body(*args):
        operands = list(args)
        if partition_name is not None:
            operands.append(bass2jax.partition_id_tensor())
        return tuple(
            bass2jax._bass_exec_p.bind(
                *operands,
                out_avals=tuple(out_avals),
                in_names=tuple(all_in_names),
                out_names=tuple(out_names),
                lowering_input_output_aliases=(),
                sim_require_finite=True,
                sim_require_nnan=True,
                nc=nc,
            )
        )

    devices = jax.devices()[:N_CORES]
    mesh = Mesh(np.asarray(devices), ("core",))
    n_args = n_params + len(out_names)
    sharded = jax.jit(
        shard_map(
            _body,
            mesh=mesh,
            in_specs=(PartitionSpec("core"),) * n_args,
            out_specs=(PartitionSpec("core"),) * len(out_names),
            check_rep=False,
        ),
        keep_unused=True,
    )
    shard = NamedSharding(mesh, PartitionSpec("core"))
    zero_dev = [
        jax.device_put(
            np.zeros((N_CORES * z.shape[0], *z.shape[1:]), z.dtype), shard
        )
        for z in zero_outs
    ]
    cv_dev = jax.device_put(
        np.concatenate([_consts()] * N_CORES, axis=0), shard
    )

    def run(x_full: np.ndarray):
        args = []
        for n in in_names:
            if n == "x":
                args.append(jax.device_put(x_full, shard))
            elif n == "consts":
                args.append(cv_dev)
            else:
                raise KeyError(n)
        outs = sharded(*args, *zero_dev)
        return {
            n: np.asarray(outs[i]).reshape(N_CORES, *out_avals[i].shape)
            for i, n in enumerate(out_names)
        }

    return run


def kernel(inputs: np.ndarray, **_ignored):
    global _RUNNER_CACHE
    if _RUNNER_CACHE is None:
        _RUNNER_CACHE = _build_runner()

    inputs = np.ascontiguousarray(inputs, dtype=np.float32)
    res = _RUNNER_CACHE(inputs)
    downsampled = res["out"].reshape(B, HH, WW, C)
    max_norm_index = np.int32(res["idx"][0, 0, 0])
    return downsampled, max_norm_index


# revision 24
# speedup vs baseline: 1.0079x; 1.0079x over previous
"""APS (adaptive polyphase sampling) layer kernel for Trainium2, 8 NeuronCores.

Problem: inputs [32,128,128,128] f32. Split into 4 polyphase components
comps[k] = inputs[:, i::2, j::2, :] (k = i*2+j). Compute L2 norm of each
component over all elements, pick argmax k*, output (comps[k*], k*).

Strategy (data-parallel over batch, 8 cores x 4 batches):
  - Each core streams its 32MB shard as 8 contiguous-run tiles of 4MB
    ([128 partitions = 2 batches x 64 rows of one h-parity] x [32KB free]).
  - ScalarE squares each tile with accum_out -> per-(h-parity, w-parity)
    partial sums; VectorE reduces; AllReduce over the 8 cores; argmax and
    the (i,j) offsets are computed on-device.
  - The winning component is copied with one dynamic-offset DRAM->DRAM DMA
    per core (strided read, contiguous write).
"""

import numpy as np

import concourse.bass as bass
from concourse import mybir

N_CORES = 8
B, H, W, C = 32, 128, 128, 128
BS = B // N_CORES          # batches per core
HH, WW = H // 2, W // 2
NT = 8                     # tiles per core: (ii=2) x (b2=2) x (wq=2)
NBUF = 4                   # tile ring depth (hint tiles stay resident)
TILE_F = 64 * C            # free elems per tile partition-line (wl*c) = 8192
HALF_F = TILE_F // 2       # per parity-j slice = 4096

# Speculative-output hint: component whose data is written to `out` during
# phase A (overlapped with the input streaming). If the argmax turns out
# different, a conditional rewrite fixes `out` (correct for any input,
# fastest when the hint hits).
K_HINT = 3
I_HINT, J_HINT = K_HINT // 2, K_HINT % 2

F32 = mybir.dt.float32
I32 = mybir.dt.int32

def _build(variant: str = "full"):
    # variant "full": collective allreduce + branchy gather (correct kernel)
    # variant "sim":  no collective, static gather (TimelineSim-compatible)
    nc = bass.Bass("TRN2", num_devices=N_CORES)

    x = nc.dram_tensor("x", [BS, H, W, C], F32, kind="ExternalInput")
    cvec_d = nc.dram_tensor("consts", [1, 16], F32, kind="ExternalInput")
    out = nc.dram_tensor("out", [BS, HH, WW, C], F32, kind="ExternalOutput")
    idx_out = nc.dram_tensor("idx", [1, 1], I32, kind="ExternalOutput")

    cc_in = nc.dram_tensor("cc_in", [1, 4], F32)
    cc_out = nc.dram_tensor("cc_out", [1, 4], F32, addr_space="Shared")

    # tile (ii, b2, wq): partitions=(bl,hh), free=(wl,c); all strides
    # merge into [128 x 32KB-contiguous] DMA lines.
    x_t = x[:].rearrange(
        "(b2 bl) (hh ii) (wq wl) c -> ii b2 wq (bl hh) (wl c)",
        b2=2, ii=2, wq=2,
    )
    # final gather view: [ii, jj, b, hh, ww, c]
    x_g = x[:].rearrange("b (hh ii) (ww jj) c -> ii jj b hh ww c", ii=2, jj=2)
    # speculative-write destination view: [b2, wq, (bl hh), (wl c)]
    out_sp = out[:].rearrange(
        "(b2 bl) hh (wq wl) c -> b2 wq (bl hh) (wl c)", b2=2, wq=2,
    )
    # load order: hint-parity tiles LAST — with NBUF=4 they stay resident in
    # the ring, and their speculative output writes are queued after all
    # loads, draining while the compute tail runs.
    tiles = [(1 - I_HINT, b2, wq) for b2 in range(2) for wq in range(2)]
    tiles += [(I_HINT, b2, wq) for b2 in range(2) for wq in range(2)]

    from contextlib import ExitStack

    with ExitStack() as ctx:
        e = ctx.enter_context
        tbuf = e(nc.sbuf_tensor([128, NBUF * TILE_F], F32))
        scratch = e(nc.sbuf_tensor([128, TILE_F], F32))
        acc = e(nc.sbuf_tensor([128, 16], F32))
        redT = e(nc.sbuf_tensor([1, 512], F32))
        red4 = e(nc.sbuf_tensor([128, 4], F32))
        norms4 = e(nc.sbuf_tensor([1, 4], F32))
        normsG = e(nc.sbuf_tensor([1, 4], F32))
        onehot = e(nc.sbuf_tensor([1, 4], F32))
        junk = e(nc.sbuf_tensor([1, 4], F32))
        maxv = e(nc.sbuf_tensor([1, 1], F32))
        kf = e(nc.sbuf_tensor([1, 1], F32))
        ki = e(nc.sbuf_tensor([1, 1], I32))
        cvec = e(nc.sbuf_tensor([1, 16], F32))
        s_load = e(nc.semaphore("s_load"))
        s_const = e(nc.semaphore("s_const"))
        s_act = e(nc.semaphore("s_act"))
        s_dve = e(nc.semaphore("s_dve"))
        s_flat = e(nc.semaphore("s_flat"))
        s_ccin = e(nc.semaphore("s_ccin"))
        s_cc = e(nc.semaphore("s_cc"))
        s_ccout = e(nc.semaphore("s_ccout"))
        s_idx = e(nc.semaphore("s_idx"))
        s_out = e(nc.semaphore("s_out"))
        s_spec = e(nc.semaphore("s_spec"))
        s_chain = e(nc.semaphore("s_chain"))
        block = e(nc.Block())
        @block.gpsimd
        def _(gp):
            spec = variant != "sim_nogather"
            for t in range(NT):
                ii, b2, wq = tiles[t]
                if t >= NBUF:
                    gp.wait_ge(s_act, t - NBUF + 1)
                slot = t % NBUF
                gp.dma_start(
                    out=tbuf[:, slot * TILE_F:(slot + 1) * TILE_F],
                    in_=x_t[ii, b2, wq],
                ).then_inc(s_load, 16)
            # consts are only needed by the argmax chain near the end; keep
            # this DMA out of the loads' way
            gp.dma_start(out=cvec[:], in_=cvec_d[:]).then_inc(s_const, 16)
            if spec:
                # speculative output writes: J_HINT-parity w-columns of the
                # (still-resident) hint tiles -> out (strided SBUF read,
                # contiguous DRAM write). Queued after all loads on the same
                # SWDGE queue: each SDMA engine drains its ring in order, so
                # these reads see the loads' writes, and they drain while the
                # compute tail runs.
                for t in range(NT):
                    ii, b2, wq = tiles[t]
                    if ii != I_HINT:
                        continue
                    slot = t % NBUF
                    sbv = tbuf[
                        :, slot * TILE_F:(slot + 1) * TILE_F
                    ].rearrange("p (wl2 j c) -> p j wl2 c", j=2, c=C)
                    gp.dma_start(
                        out=out_sp[b2, wq], in_=sbv[:, J_HINT],
                    ).then_inc(s_spec, 16)

            # cross-partition flatten of per-partition accumulators
            gp.wait_ge(s_dve, 1)
            gp.dma_start(out=redT[:], in_=red4[:]).then_inc(s_flat, 16)

            # AllReduce the 4 partial sums across cores
            if variant == "full":
                gp.wait_ge(s_dve, 2)
                gp.dma_start(out=cc_in[:], in_=norms4[:]).then_inc(s_ccin, 16)
                gp.wait_ge(s_ccin, 16)
                gp.collective_compute(
                    "AllReduce",
                    mybir.AluOpType.add,
                    replica_groups=[list(range(N_CORES))],
                    ins=[cc_in[:]],
                    outs=[cc_out[:]],
                ).then_inc(s_cc)
                gp.wait_ge(s_cc, 1)
                gp.dma_start(out=normsG[:], in_=cc_out[:]).then_inc(s_ccout, 16)

            # index output + conditional rewrite when the hint missed
            gp.wait_ge(s_dve, 3)
            gp.dma_start(out=idx_out[:], in_=ki[:]).then_inc(s_idx, 16)
            if variant == "full":
                kv = nc.values_load(
                    ki[0:1, 0:1], engines=(mybir.EngineType.Pool,),
                    min_val=0, max_val=3, skip_runtime_bounds_check=True,
                )
                for kk in range(4):
                    if kk == K_HINT:
                        continue  # hint hit: speculative writes already did it
                    cond = nc.snap(
                        kv, engines=bass.OrderedSet([mybir.EngineType.Pool])
                    ) == kk
                    with gp.If(cond):
                        # all speculative writes must have landed before the
                        # rewrite touches the same addresses
                        gp.wait_ge(s_spec, 16 * 4)
                        for b in range(BS):
                            gp.dma_start(
                                out=out[b], in_=x_g[kk // 2, kk % 2, b],
                            ).then_inc(s_out, 16)
                        gp.wait_ge(s_out, 16 * BS)
            gp.wait_ge(s_idx, 16)
            if variant != "sim_nogather":
                gp.wait_ge(s_spec, 16 * 4)

        @block.scalar
        def _(act):
            for t in range(NT):
                ii, b2, wq = tiles[t]
                act.wait_ge(s_load, 16 * (t + 1))
                slot = t % NBUF
                sb = tbuf[:, slot * TILE_F:(slot + 1) * TILE_F].rearrange(
                    "p (wl2 j c) -> p j wl2 c", j=2, c=C,
                )
                for j in range(2):
                    s = j * 8 + ii * 4 + b2 * 2 + wq
                    ins = act.activation(
                        out=scratch[:, j * HALF_F:(j + 1) * HALF_F].rearrange(
                            "p (a c) -> p a c", c=C,
                        ),
                        in_=sb[:, j],
                        func=mybir.ActivationFunctionType.Square,
                        accum_out=acc[:, s:s + 1],
                    )
                ins.then_inc(s_act, 1)

        @block.vector
        def _(dve):
            # acc slots: s = j*8 + ii*4 + r  ->  [p, (j ii r)] -> sum r
            dve.wait_ge(s_act, NT)
            dve.tensor_reduce(
                out=red4[:],
                in_=acc[:].rearrange("p (j ii r) -> p ii j r", j=2, ii=2),
                axis=mybir.AxisListType.X,
                op=mybir.AluOpType.add,
            ).then_inc(s_dve, 1)

            # redT[0, p*4 + k] -> norms4[0, k]
            dve.wait_ge(s_flat, 16)
            dve.tensor_reduce(
                out=norms4[:],
                in_=redT[:].rearrange("q (p k) -> q k p", k=4),
                axis=mybir.AxisListType.X,
                op=mybir.AluOpType.add,
            ).then_inc(s_dve, 1)

            # argmax arithmetic on the globally-reduced norms
            # NB: consecutive dependent ops on the same engine need explicit
            # waits (deep pipeline; no same-engine RAW interlock).
            cb = 0
            if variant == "full":
                dve.wait_ge(s_ccout, 16)
            else:
                dve.wait_ge(s_dve, 2)
                dve.tensor_copy(out=normsG[:], in_=norms4[:]).then_inc(s_chain, 1)
                dve.wait_ge(s_chain, 1)
                cb = 1
            dve.wait_ge(s_const, 16)
            dve.tensor_reduce(
                out=maxv[:], in_=normsG[:],
                axis=mybir.AxisListType.X, op=mybir.AluOpType.max,
            ).then_inc(s_chain, 1)
            dve.wait_ge(s_chain, cb + 1)
            dve.tensor_scalar(
                out=onehot[:], in0=normsG[:],
                scalar1=maxv[0:1, 0:1], scalar2=None,
                op0=mybir.AluOpType.is_ge,
            ).then_inc(s_chain, 1)
            dve.wait_ge(s_chain, cb + 2)
            dve.tensor_tensor(
                out=junk[:], in0=onehot[:], in1=cvec[0:1, 0:4],
                op=mybir.AluOpType.mult,
            ).then_inc(s_chain, 1)
            dve.wait_ge(s_chain, cb + 3)
            dve.tensor_reduce(
                out=kf[:], in_=junk[:],
                axis=mybir.AxisListType.X, op=mybir.AluOpType.add,
            ).then_inc(s_chain, 1)
            dve.wait_ge(s_chain, cb + 4)
            dve.tensor_copy(out=ki[:], in_=kf[:]).then_inc(s_dve, 1)

    return nc


def _consts() -> np.ndarray:
    c = np.zeros((1, 16), dtype=np.float32)
    c[0, 0:4] = [0, 1, 2, 3]    # k
    c[0, 4:8] = [0, 0, 1, 1]    # i = k // 2
    c[0, 8:12] = [0, 1, 0, 1]   # j = k % 2
    return c


_RUNNER_CACHE = None


def _build_runner():
    """Compile the bass module once and return a reusable jitted executor
    (mirrors bass2jax.run_bass_via_pjrt's multi-core path, minus donation,
    so repeat kernel() calls skip re-lowering)."""
    import jax
    from jax.experimental.shard_map import shard_map
    from jax.sharding import Mesh, NamedSharding, PartitionSpec

    from concourse import bass2jax

    nc = _build("full")
    bass2jax.install_neuronx_cc_hook()
    partition_name = nc.partition_id_tensor.name if nc.partition_id_tensor else None

    in_names, out_names, out_avals, zero_outs = [], [], [], []
    for alloc in nc.m.functions[0].allocations:
        if not isinstance(alloc, mybir.MemoryLocationSet):
            continue
        name = alloc.memorylocations[0].name
        if alloc.kind == "ExternalInput":
            if name != partition_name:
                in_names.append(name)
        elif alloc.kind == "ExternalOutput":
            shape = tuple(alloc.tensor_shape)
            dtype = mybir.dt.np(alloc.dtype)
            out_avals.append(jax.core.ShapedArray(shape, dtype))
            out_names.append(name)
            zero_outs.append(np.zeros(shape, dtype))

    n_params = len(in_names)
    all_in_names = list(in_names) + list(out_names)
    if partition_name is not None:
        all_in_names.append(partition_name)

    def _body(*args):
        operands = list(args)
        if partition_name is not None:
            operands.append(bass2jax.partition_id_tensor())
        return tuple(
            bass2jax._bass_exec_p.bind(
                *operands,
                out_avals=tuple(out_avals),
                in_names=tuple(all_in_names),
                out_names=tuple(out_names),
                lowering_input_output_aliases=(),
                sim_require_finite=True,
                sim_require_nnan=True,
                nc=nc,
            )
        )

    devices = jax.devices()[:N_CORES]
    mesh = Mesh(np.asarray(devices), ("core",))
    n_args = n_params + len(out_names)
    sharded = jax.jit(
        shard_map(
            _body,
            mesh=mesh,
            in_specs=(PartitionSpec("core"),) * n_args,
            out_specs=(PartitionSpec("core"),) * len(out_names),
            check_rep=False,
        ),
        keep_unused=True,
    )
    shard = NamedSharding(mesh, PartitionSpec("core"))
    zero_dev = [
        jax.device_put(
            np.zeros((N_CORES * z.shape[0], *z.shape[1:]), z.dtype), shard
        )
        for z in zero_outs
    ]
    cv_dev = jax.device_put(
        np.concatenate([_consts()] * N_CORES, axis=0), shard
    )

    def run(x_full: np.ndarray):
        args = []
        for n in in_names:
            if n == "x":
                args.append(jax.device_put(x_full, shard))
            elif n == "consts":
                args.append(cv_dev)
            else:
                raise KeyError(n)
        outs = sharded(*args, *zero_dev)
        return {
            n: np.asarray(outs[i]).reshape(N_CORES, *out_avals[i].shape)
            for i, n in enumerate(out_names)
        }

    return run


def kernel(inputs: np.ndarray, **_ignored):
    global _RUNNER_CACHE
    if _RUNNER_CACHE is None:
        _RUNNER_CACHE = _build_runner()

    inputs = np.ascontiguousarray(inputs, dtype=np.float32)
    res = _RUNNER_CACHE(inputs)
    downsampled = res["out"].reshape(B, HH, WW, C)
    max_norm_index = np.int32(res["idx"][0, 0, 0])
    return downsampled, max_norm_index


# revision 25
# speedup vs baseline: 1.0082x; 1.0003x over previous
"""APS (adaptive polyphase sampling) layer kernel for Trainium2, 8 NeuronCores.

Problem: inputs [32,128,128,128] f32. Split into 4 polyphase components
comps[k] = inputs[:, i::2, j::2, :] (k = i*2+j). Compute L2 norm of each
component over all elements, pick argmax k*, output (comps[k*], k*).

Strategy (data-parallel over batch, 8 cores x 4 batches):
  - Each core streams its 32MB shard as 8 contiguous-run tiles of 4MB
    ([128 partitions = 2 batches x 64 rows of one h-parity] x [32KB free]).
  - ScalarE squares each tile with accum_out -> per-(h-parity, w-parity)
    partial sums; VectorE reduces; AllReduce over the 8 cores; argmax and
    the (i,j) offsets are computed on-device.
  - The winning component is copied with one dynamic-offset DRAM->DRAM DMA
    per core (strided read, contiguous write).
"""

import numpy as np

import concourse.bass as bass
from concourse import mybir

N_CORES = 8
B, H, W, C = 32, 128, 128, 128
BS = B // N_CORES          # batches per core
HH, WW = H // 2, W // 2
NT = 8                     # tiles per core: (ii=2) x (b2=2) x (wq=2)
NBUF = 4                   # tile ring depth (hint tiles stay resident)
TILE_F = 64 * C            # free elems per tile partition-line (wl*c) = 8192
HALF_F = TILE_F // 2       # per parity-j slice = 4096

# Speculative-output hint: component whose data is written to `out` during
# phase A (overlapped with the input streaming). If the argmax turns out
# different, a conditional rewrite fixes `out` (correct for any input,
# fastest when the hint hits).
K_HINT = 3
I_HINT, J_HINT = K_HINT // 2, K_HINT % 2

F32 = mybir.dt.float32
I32 = mybir.dt.int32

def _build(variant: str = "full"):
    # variant "full": collective allreduce + branchy gather (correct kernel)
    # variant "sim":  no collective, static gather (TimelineSim-compatible)
    nc = bass.Bass("TRN2", num_devices=N_CORES)

    x = nc.dram_tensor("x", [BS, H, W, C], F32, kind="ExternalInput")
    cvec_d = nc.dram_tensor("consts", [1, 16], F32, kind="ExternalInput")
    out = nc.dram_tensor("out", [BS, HH, WW, C], F32, kind="ExternalOutput")
    idx_out = nc.dram_tensor("idx", [1, 1], I32, kind="ExternalOutput")

    cc_in = nc.dram_tensor("cc_in", [1, 4], F32)
    cc_out = nc.dram_tensor("cc_out", [1, 4], F32, addr_space="Shared")

    # tile (ii, b2, wq): partitions=(bl,hh), free=(wl,c); all strides
    # merge into [128 x 32KB-contiguous] DMA lines.
    x_t = x[:].rearrange(
        "(b2 bl) (hh ii) (wq wl) c -> ii b2 wq (bl hh) (wl c)",
        b2=2, ii=2, wq=2,
    )
    # final gather view: [ii, jj, b, hh, ww, c]
    x_g = x[:].rearrange("b (hh ii) (ww jj) c -> ii jj b hh ww c", ii=2, jj=2)
    # speculative-write destination view: [b2, wq, (bl hh), (wl c)]
    out_sp = out[:].rearrange(
        "(b2 bl) hh (wq wl) c -> b2 wq (bl hh) (wl c)", b2=2, wq=2,
    )
    # load order: hint-parity tiles LAST — with NBUF=4 they stay resident in
    # the ring, and their speculative output writes are queued after all
    # loads, draining while the compute tail runs.
    tiles = [(1 - I_HINT, b2, wq) for b2 in range(2) for wq in range(2)]
    tiles += [(I_HINT, b2, wq) for b2 in range(2) for wq in range(2)]

    from contextlib import ExitStack

    with ExitStack() as ctx:
        e = ctx.enter_context
        tbuf = e(nc.sbuf_tensor([128, NBUF * TILE_F], F32))
        scratch = e(nc.sbuf_tensor([128, TILE_F], F32))
        acc = e(nc.sbuf_tensor([128, 16], F32))
        redT = e(nc.sbuf_tensor([1, 512], F32))
        red4 = e(nc.sbuf_tensor([128, 4], F32))
        norms4 = e(nc.sbuf_tensor([1, 4], F32))
        normsG = e(nc.sbuf_tensor([1, 4], F32))
        onehot = e(nc.sbuf_tensor([1, 4], F32))
        junk = e(nc.sbuf_tensor([1, 4], F32))
        maxv = e(nc.sbuf_tensor([1, 1], F32))
        kf = e(nc.sbuf_tensor([1, 1], F32))
        ki = e(nc.sbuf_tensor([1, 1], I32))
        cvec = e(nc.sbuf_tensor([1, 16], F32))
        s_load = e(nc.semaphore("s_load"))
        s_const = e(nc.semaphore("s_const"))
        s_act = e(nc.semaphore("s_act"))
        s_dve = e(nc.semaphore("s_dve"))
        s_flat = e(nc.semaphore("s_flat"))
        s_ccin = e(nc.semaphore("s_ccin"))
        s_cc = e(nc.semaphore("s_cc"))
        s_ccout = e(nc.semaphore("s_ccout"))
        s_idx = e(nc.semaphore("s_idx"))
        s_out = e(nc.semaphore("s_out"))
        s_spec = e(nc.semaphore("s_spec"))
        s_chain = e(nc.semaphore("s_chain"))
        # All DMAs are semaphore-verified complete before the block ends, so
        # GpSimd's expensive exit-time DGE drain is redundant.
        block = e(nc.Block(no_gpsimd_drain=True))
        @block.gpsimd
        def _(gp):
            spec = variant != "sim_nogather"
            for t in range(NT):
                ii, b2, wq = tiles[t]
                if t >= NBUF:
                    gp.wait_ge(s_act, t - NBUF + 1)
                slot = t % NBUF
                gp.dma_start(
                    out=tbuf[:, slot * TILE_F:(slot + 1) * TILE_F],
                    in_=x_t[ii, b2, wq],
                ).then_inc(s_load, 16)
            # consts are only needed by the argmax chain near the end; keep
            # this DMA out of the loads' way
            gp.dma_start(out=cvec[:], in_=cvec_d[:]).then_inc(s_const, 16)
            if spec:
                # speculative output writes: J_HINT-parity w-columns of the
                # (still-resident) hint tiles -> out (strided SBUF read,
                # contiguous DRAM write). Queued after all loads on the same
                # SWDGE queue: each SDMA engine drains its ring in order, so
                # these reads see the loads' writes, and they drain while the
                # compute tail runs.
                for t in range(NT):
                    ii, b2, wq = tiles[t]
                    if ii != I_HINT:
                        continue
                    slot = t % NBUF
                    sbv = tbuf[
                        :, slot * TILE_F:(slot + 1) * TILE_F
                    ].rearrange("p (wl2 j c) -> p j wl2 c", j=2, c=C)
                    gp.dma_start(
                        out=out_sp[b2, wq], in_=sbv[:, J_HINT],
                    ).then_inc(s_spec, 16)

            # cross-partition flatten of per-partition accumulators
            gp.wait_ge(s_dve, 1)
            gp.dma_start(out=redT[:], in_=red4[:]).then_inc(s_flat, 16)

            # AllReduce the 4 partial sums across cores
            if variant == "full":
                gp.wait_ge(s_dve, 2)
                gp.dma_start(out=cc_in[:], in_=norms4[:]).then_inc(s_ccin, 16)
                gp.wait_ge(s_ccin, 16)
                gp.collective_compute(
                    "AllReduce",
                    mybir.AluOpType.add,
                    replica_groups=[list(range(N_CORES))],
                    ins=[cc_in[:]],
                    outs=[cc_out[:]],
                ).then_inc(s_cc)
                gp.wait_ge(s_cc, 1)
                gp.dma_start(out=normsG[:], in_=cc_out[:]).then_inc(s_ccout, 16)

            # index output + conditional rewrite when the hint missed
            gp.wait_ge(s_dve, 3)
            gp.dma_start(out=idx_out[:], in_=ki[:]).then_inc(s_idx, 16)
            if variant == "full":
                kv = nc.values_load(
                    ki[0:1, 0:1], engines=(mybir.EngineType.Pool,),
                    min_val=0, max_val=3, skip_runtime_bounds_check=True,
                )
                for kk in range(4):
                    if kk == K_HINT:
                        continue  # hint hit: speculative writes already did it
                    cond = nc.snap(
                        kv, engines=bass.OrderedSet([mybir.EngineType.Pool])
                    ) == kk
                    with gp.If(cond):
                        # all speculative writes must have landed before the
                        # rewrite touches the same addresses
                        gp.wait_ge(s_spec, 16 * 4)
                        for b in range(BS):
                            gp.dma_start(
                                out=out[b], in_=x_g[kk // 2, kk % 2, b],
                            ).then_inc(s_out, 16)
                        gp.wait_ge(s_out, 16 * BS)
            gp.wait_ge(s_idx, 16)
            if variant != "sim_nogather":
                gp.wait_ge(s_spec, 16 * 4)

        @block.scalar
        def _(act):
            for t in range(NT):
                ii, b2, wq = tiles[t]
                act.wait_ge(s_load, 16 * (t + 1))
                slot = t % NBUF
                sb = tbuf[:, slot * TILE_F:(slot + 1) * TILE_F].rearrange(
                    "p (wl2 j c) -> p j wl2 c", j=2, c=C,
                )
                for j in range(2):
                    s = j * 8 + ii * 4 + b2 * 2 + wq
                    ins = act.activation(
                        out=scratch[:, j * HALF_F:(j + 1) * HALF_F].rearrange(
                            "p (a c) -> p a c", c=C,
                        ),
                        in_=sb[:, j],
                        func=mybir.ActivationFunctionType.Square,
                        accum_out=acc[:, s:s + 1],
                    )
                ins.then_inc(s_act, 1)

        @block.vector
        def _(dve):
            # acc slots: s = j*8 + ii*4 + r  ->  [p, (j ii r)] -> sum r
            dve.wait_ge(s_act, NT)
            dve.tensor_reduce(
                out=red4[:],
                in_=acc[:].rearrange("p (j ii r) -> p ii j r", j=2, ii=2),
                axis=mybir.AxisListType.X,
                op=mybir.AluOpType.add,
            ).then_inc(s_dve, 1)

            # redT[0, p*4 + k] -> norms4[0, k]
            dve.wait_ge(s_flat, 16)
            dve.tensor_reduce(
                out=norms4[:],
                in_=redT[:].rearrange("q (p k) -> q k p", k=4),
                axis=mybir.AxisListType.X,
                op=mybir.AluOpType.add,
            ).then_inc(s_dve, 1)

            # argmax arithmetic on the globally-reduced norms
            # NB: consecutive dependent ops on the same engine need explicit
            # waits (deep pipeline; no same-engine RAW interlock).
            cb = 0
            if variant == "full":
                dve.wait_ge(s_ccout, 16)
            else:
                dve.wait_ge(s_dve, 2)
                dve.tensor_copy(out=normsG[:], in_=norms4[:]).then_inc(s_chain, 1)
                dve.wait_ge(s_chain, 1)
                cb = 1
            dve.wait_ge(s_const, 16)
            dve.tensor_reduce(
                out=maxv[:], in_=normsG[:],
                axis=mybir.AxisListType.X, op=mybir.AluOpType.max,
            ).then_inc(s_chain, 1)
            dve.wait_ge(s_chain, cb + 1)
            dve.tensor_scalar(
                out=onehot[:], in0=normsG[:],
                scalar1=maxv[0:1, 0:1], scalar2=None,
                op0=mybir.AluOpType.is_ge,
            ).then_inc(s_chain, 1)
            dve.wait_ge(s_chain, cb + 2)
            dve.tensor_tensor(
                out=junk[:], in0=onehot[:], in1=cvec[0:1, 0:4],
                op=mybir.AluOpType.mult,
            ).then_inc(s_chain, 1)
            dve.wait_ge(s_chain, cb + 3)
            dve.tensor_reduce(
                out=kf[:], in_=junk[:],
                axis=mybir.AxisListType.X, op=mybir.AluOpType.add,
            ).then_inc(s_chain, 1)
            dve.wait_ge(s_chain, cb + 4)
            dve.tensor_copy(out=ki[:], in_=kf[:]).then_inc(s_dve, 1)

    return nc


def _consts() -> np.ndarray:
    c = np.zeros((1, 16), dtype=np.float32)
    c[0, 0:4] = [0, 1, 2, 3]    # k
    c[0, 4:8] = [0, 0, 1, 1]    # i = k // 2
    c[0, 8:12] = [0, 1, 0, 1]   # j = k % 2
    return c


_RUNNER_CACHE = None


def _build_runner():
    """Compile the bass module once and return a reusable jitted executor
    (mirrors bass2jax.run_bass_via_pjrt's multi-core path, minus donation,
    so repeat kernel() calls skip re-lowering)."""
    import jax
    from jax.experimental.shard_map import shard_map
    from jax.sharding import Mesh, NamedSharding, PartitionSpec

    from concourse import bass2jax

    nc = _build("full")
    bass2jax.install_neuronx_cc_hook()
    partition_name = nc.partition_id_tensor.name if nc.partition_id_tensor else None

    in_names, out_names, out_avals, zero_outs = [], [], [], []
    for alloc in nc.m.functions[0].allocations:
        if not isinstance(alloc, mybir.MemoryLocationSet):
            continue
        name = alloc.memorylocations[0].name
        if alloc.kind == "ExternalInput":
            if name != partition_name:
                in_names.append(name)
        elif alloc.kind == "ExternalOutput":
            shape = tuple(alloc.tensor_shape)
            dtype = mybir.dt.np(alloc.dtype)
            out_avals.append(jax.core.ShapedArray(shape, dtype))
            out_names.append(name)
            zero_outs.append(np.zeros(shape, dtype))

    n_params = len(in_names)
    all_in_names = list(in_names) + list(out_names)
    if partition_name is not None:
        all_in_names.append(partition_name)

    def _body(*args):
        operands = list(args)
        if partition_name is not None:
            operands.append(bass2jax.partition_id_tensor())
        return tuple(
            bass2jax._bass_exec_p.bind(
                *operands,
                out_avals=tuple(out_avals),
                in_names=tuple(all_in_names),
                out_names=tuple(out_names),
                lowering_input_output_aliases=(),
                sim_require_finite=True,
                sim_require_nnan=True,
                nc=nc,
            )
        )

    devices = jax.devices()[:N_CORES]
    mesh = Mesh(np.asarray(devices), ("core",))
    n_args = n_params + len(out_names)
    sharded = jax.jit(
        shard_map(
            _body,
            mesh=mesh,
            in_specs=(PartitionSpec("core"),) * n_args,
            out_specs=(PartitionSpec("core"),) * len(out_names),
            check_rep=False,
        ),
        keep_unused=True,
    )
    shard = NamedSharding(mesh, PartitionSpec("core"))
    zero_dev = [
        jax.device_put(
            np.zeros((N_CORES * z.shape[0], *z.shape[1:]), z.dtype), shard
        )
        for z in zero_outs
    ]
    cv_dev = jax.device_put(
        np.concatenate([_consts()] * N_CORES, axis=0), shard
    )

    def run(x_full: np.ndarray):
        args = []
        for n in in_names:
            if n == "x":
                args.append(jax.device_put(x_full, shard))
            elif n == "consts":
                args.append(cv_dev)
            else:
                raise KeyError(n)
        outs = sharded(*args, *zero_dev)
        return {
            n: np.asarray(outs[i]).reshape(N_CORES, *out_avals[i].shape)
            for i, n in enumerate(out_names)
        }

    return run


def kernel(inputs: np.ndarray, **_ignored):
    global _RUNNER_CACHE
    if _RUNNER_CACHE is None:
        _RUNNER_CACHE = _build_runner()

    inputs = np.ascontiguousarray(inputs, dtype=np.float32)
    res = _RUNNER_CACHE(inputs)
    downsampled = res["out"].reshape(B, HH, WW, C)
    max_norm_index = np.int32(res["idx"][0, 0, 0])
    return downsampled, max_norm_index


# revision 27
# speedup vs baseline: 1.0103x; 1.0021x over previous
"""APS (adaptive polyphase sampling) layer kernel for Trainium2, 8 NeuronCores.

Problem: inputs [32,128,128,128] f32. Split into 4 polyphase components
comps[k] = inputs[:, i::2, j::2, :] (k = i*2+j). Compute L2 norm of each
component over all elements, pick argmax k*, output (comps[k*], k*).

Strategy (data-parallel over batch, 8 cores x 4 batches):
  - Each core streams its 32MB shard as 8 contiguous-run tiles of 4MB
    ([128 partitions = 2 batches x 64 rows of one h-parity] x [32KB free]).
  - ScalarE squares each tile with accum_out -> per-(h-parity, w-parity)
    partial sums; VectorE reduces; AllReduce over the 8 cores; argmax and
    the (i,j) offsets are computed on-device.
  - The winning component is copied with one dynamic-offset DRAM->DRAM DMA
    per core (strided read, contiguous write).
"""

import numpy as np

import concourse.bass as bass
from concourse import mybir

N_CORES = 8
B, H, W, C = 32, 128, 128, 128
BS = B // N_CORES          # batches per core
HH, WW = H // 2, W // 2
NT = 8                     # tiles per core: (ii=2) x (b2=2) x (wq=2)
NBUF = 4                   # tile ring depth (hint tiles stay resident)
TILE_F = 64 * C            # free elems per tile partition-line (wl*c) = 8192
HALF_F = TILE_F // 2       # per parity-j slice = 4096

# Speculative-output hint: component whose data is written to `out` during
# phase A (overlapped with the input streaming). If the argmax turns out
# different, a conditional rewrite fixes `out` (correct for any input,
# fastest when the hint hits).
K_HINT = 3
I_HINT, J_HINT = K_HINT // 2, K_HINT % 2

F32 = mybir.dt.float32
I32 = mybir.dt.int32

def _build(variant: str = "full"):
    # variant "full": collective allreduce + branchy gather (correct kernel)
    # variant "sim":  no collective, static gather (TimelineSim-compatible)
    nc = bass.Bass("TRN2", num_devices=N_CORES)

    x = nc.dram_tensor("x", [BS, H, W, C], F32, kind="ExternalInput")
    cvec_d = nc.dram_tensor("consts", [1, 16], F32, kind="ExternalInput")
    out = nc.dram_tensor("out", [BS, HH, WW, C], F32, kind="ExternalOutput")
    idx_out = nc.dram_tensor("idx", [1, 1], I32, kind="ExternalOutput")

    cc_in = nc.dram_tensor("cc_in", [1, 4], F32)
    cc_out = nc.dram_tensor("cc_out", [1, 4], F32, addr_space="Shared")

    # tile (ii, b2, wq): partitions=(bl,hh), free=(wl,c); all strides
    # merge into [128 x 32KB-contiguous] DMA lines.
    x_t = x[:].rearrange(
        "(b2 bl) (hh ii) (wq wl) c -> ii b2 wq (bl hh) (wl c)",
        b2=2, ii=2, wq=2,
    )
    # final gather view: [ii, jj, b, hh, ww, c]
    x_g = x[:].rearrange("b (hh ii) (ww jj) c -> ii jj b hh ww c", ii=2, jj=2)
    # speculative-write destination view: [b2, wq, (bl hh), (wl c)]
    out_sp = out[:].rearrange(
        "(b2 bl) hh (wq wl) c -> b2 wq (bl hh) (wl c)", b2=2, wq=2,
    )
    # load order: hint-parity tiles LAST — with NBUF=4 they stay resident in
    # the ring, and their speculative output writes are queued after all
    # loads, draining while the compute tail runs.
    tiles = [(1 - I_HINT, b2, wq) for b2 in range(2) for wq in range(2)]
    tiles += [(I_HINT, b2, wq) for b2 in range(2) for wq in range(2)]

    from contextlib import ExitStack

    with ExitStack() as ctx:
        e = ctx.enter_context
        tbuf = e(nc.sbuf_tensor([128, NBUF * TILE_F], F32))
        scratch = e(nc.sbuf_tensor([128, TILE_F], F32))
        acc = e(nc.sbuf_tensor([128, 16], F32))
        redT = e(nc.sbuf_tensor([1, 512], F32))
        red4 = e(nc.sbuf_tensor([128, 4], F32))
        norms4 = e(nc.sbuf_tensor([1, 4], F32))
        normsG = e(nc.sbuf_tensor([1, 4], F32))
        onehot = e(nc.sbuf_tensor([1, 4], F32))
        junk = e(nc.sbuf_tensor([1, 4], F32))
        maxv = e(nc.sbuf_tensor([1, 1], F32))
        kf = e(nc.sbuf_tensor([1, 1], F32))
        ki = e(nc.sbuf_tensor([1, 1], I32))
        cvec = e(nc.sbuf_tensor([1, 16], F32))
        s_load = e(nc.semaphore("s_load"))
        s_const = e(nc.semaphore("s_const"))
        s_act = e(nc.semaphore("s_act"))
        s_dve = e(nc.semaphore("s_dve"))
        s_flat = e(nc.semaphore("s_flat"))
        s_ccin = e(nc.semaphore("s_ccin"))
        s_cc = e(nc.semaphore("s_cc"))
        s_ccout = e(nc.semaphore("s_ccout"))
        s_idx = e(nc.semaphore("s_idx"))
        s_out = e(nc.semaphore("s_out"))
        s_spec = e(nc.semaphore("s_spec"))
        s_chain = e(nc.semaphore("s_chain"))
        # All DMAs are semaphore-verified complete before the block ends, so
        # GpSimd's expensive exit-time DGE drain is redundant.
        block = e(nc.Block(no_gpsimd_drain=True))
        @block.gpsimd
        def _(gp):
            spec = variant != "sim_nogather"
            for t in range(NT):
                ii, b2, wq = tiles[t]
                if t >= NBUF:
                    gp.wait_ge(s_act, t - NBUF + 1)
                slot = t % NBUF
                gp.dma_start(
                    out=tbuf[:, slot * TILE_F:(slot + 1) * TILE_F],
                    in_=x_t[ii, b2, wq],
                ).then_inc(s_load, 16)
            # consts are only needed by the argmax chain near the end; keep
            # this DMA out of the loads' way
            gp.dma_start(out=cvec[:], in_=cvec_d[:]).then_inc(s_const, 16)
            if spec:
                # speculative output writes: J_HINT-parity w-columns of the
                # (still-resident) hint tiles -> out (strided SBUF read,
                # contiguous DRAM write). Queued after all loads on the same
                # SWDGE queue: each SDMA engine drains its ring in order, so
                # these reads see the loads' writes, and they drain while the
                # compute tail runs.
                for t in range(NT):
                    ii, b2, wq = tiles[t]
                    if ii != I_HINT:
                        continue
                    slot = t % NBUF
                    sbv = tbuf[
                        :, slot * TILE_F:(slot + 1) * TILE_F
                    ].rearrange("p (wl2 j c) -> p j wl2 c", j=2, c=C)
                    gp.dma_start(
                        out=out_sp[b2, wq], in_=sbv[:, J_HINT],
                    ).then_inc(s_spec, 16)

            # the AllReduce itself runs on TOPSP, not the SWDGE queue, so it
            # can stay here; the small tail DMAs live on the ScalarE HWDGE
            # path instead (they would FIFO behind the 8MB of speculative
            # write descriptors on SWDGE queue 0)
            if variant == "full":
                gp.wait_ge(s_ccin, 16)
                gp.collective_compute(
                    "AllReduce",
                    mybir.AluOpType.add,
                    replica_groups=[list(range(N_CORES))],
                    ins=[cc_in[:]],
                    outs=[cc_out[:]],
                ).then_inc(s_cc)

            # conditional rewrite when the hint missed
            gp.wait_ge(s_dve, 3)
            if variant == "full":
                kv = nc.values_load(
                    ki[0:1, 0:1], engines=(mybir.EngineType.Pool,),
                    min_val=0, max_val=3, skip_runtime_bounds_check=True,
                )
                for kk in range(4):
                    if kk == K_HINT:
                        continue  # hint hit: speculative writes already did it
                    cond = nc.snap(
                        kv, engines=bass.OrderedSet([mybir.EngineType.Pool])
                    ) == kk
                    with gp.If(cond):
                        # all speculative writes must have landed before the
                        # rewrite touches the same addresses
                        gp.wait_ge(s_spec, 16 * 4)
                        for b in range(BS):
                            gp.dma_start(
                                out=out[b], in_=x_g[kk // 2, kk % 2, b],
                            ).then_inc(s_out, 16)
                        gp.wait_ge(s_out, 16 * BS)
            gp.wait_ge(s_idx, 16)
            if variant != "sim_nogather":
                gp.wait_ge(s_spec, 16 * 4)

        @block.scalar
        def _(act):
            for t in range(NT):
                ii, b2, wq = tiles[t]
                act.wait_ge(s_load, 16 * (t + 1))
                slot = t % NBUF
                sb = tbuf[:, slot * TILE_F:(slot + 1) * TILE_F].rearrange(
                    "p (wl2 j c) -> p j wl2 c", j=2, c=C,
                )
                for j in range(2):
                    s = j * 8 + ii * 4 + b2 * 2 + wq
                    ins = act.activation(
                        out=scratch[:, j * HALF_F:(j + 1) * HALF_F].rearrange(
                            "p (a c) -> p a c", c=C,
                        ),
                        in_=sb[:, j],
                        func=mybir.ActivationFunctionType.Square,
                        accum_out=acc[:, s:s + 1],
                    )
                ins.then_inc(s_act, 1)

            # tail DMAs on HWDGE (bypassing the SWDGE descriptor backlog)
            act.wait_ge(s_dve, 1)
            act.dma_start(out=redT[:], in_=red4[:]).then_inc(s_flat, 16)
            if variant == "full":
                act.wait_ge(s_dve, 2)
                act.dma_start(out=cc_in[:], in_=norms4[:]).then_inc(s_ccin, 16)
                act.wait_ge(s_cc, 1)
                act.dma_start(out=normsG[:], in_=cc_out[:]).then_inc(s_ccout, 16)
            act.wait_ge(s_dve, 3)
            act.dma_start(out=idx_out[:], in_=ki[:]).then_inc(s_idx, 16)

        @block.vector
        def _(dve):
            # acc slots: s = j*8 + ii*4 + r  ->  [p, (j ii r)] -> sum r
            dve.wait_ge(s_act, NT)
            dve.tensor_reduce(
                out=red4[:],
                in_=acc[:].rearrange("p (j ii r) -> p ii j r", j=2, ii=2),
                axis=mybir.AxisListType.X,
                op=mybir.AluOpType.add,
            ).then_inc(s_dve, 1)

            # redT[0, p*4 + k] -> norms4[0, k]
            dve.wait_ge(s_flat, 16)
            dve.tensor_reduce(
                out=norms4[:],
                in_=redT[:].rearrange("q (p k) -> q k p", k=4),
                axis=mybir.AxisListType.X,
                op=mybir.AluOpType.add,
            ).then_inc(s_dve, 1)

            # argmax arithmetic on the globally-reduced norms
            # NB: consecutive dependent ops on the same engine need explicit
            # waits (deep pipeline; no same-engine RAW interlock).
            cb = 0
            if variant == "full":
                dve.wait_ge(s_ccout, 16)
            else:
                dve.wait_ge(s_dve, 2)
                dve.tensor_copy(out=normsG[:], in_=norms4[:]).then_inc(s_chain, 1)
                dve.wait_ge(s_chain, 1)
                cb = 1
            dve.wait_ge(s_const, 16)
            dve.tensor_reduce(
                out=maxv[:], in_=normsG[:],
                axis=mybir.AxisListType.X, op=mybir.AluOpType.max,
            ).then_inc(s_chain, 1)
            dve.wait_ge(s_chain, cb + 1)
            dve.tensor_scalar(
                out=onehot[:], in0=normsG[:],
                scalar1=maxv[0:1, 0:1], scalar2=None,
                op0=mybir.AluOpType.is_ge,
            ).then_inc(s_chain, 1)
            dve.wait_ge(s_chain, cb + 2)
            dve.tensor_tensor(
                out=junk[:], in0=onehot[:], in1=cvec[0:1, 0:4],
                op=mybir.AluOpType.mult,
            ).then_inc(s_chain, 1)
            dve.wait_ge(s_chain, cb + 3)
            dve.tensor_reduce(
                out=kf[:], in_=junk[:],
                axis=mybir.AxisListType.X, op=mybir.AluOpType.add,
            ).then_inc(s_chain, 1)
            dve.wait_ge(s_chain, cb + 4)
            dve.tensor_copy(out=ki[:], in_=kf[:]).then_inc(s_dve, 1)

    return nc


def _consts() -> np.ndarray:
    c = np.zeros((1, 16), dtype=np.float32)
    c[0, 0:4] = [0, 1, 2, 3]    # k
    c[0, 4:8] = [0, 0, 1, 1]    # i = k // 2
    c[0, 8:12] = [0, 1, 0, 1]   # j = k % 2
    return c


_RUNNER_CACHE = None


def _build_runner():
    """Compile the bass module once and return a reusable jitted executor
    (mirrors bass2jax.run_bass_via_pjrt's multi-core path, minus donation,
    so repeat kernel() calls skip re-lowering)."""
    import jax
    from jax.experimental.shard_map import shard_map
    from jax.sharding import Mesh, NamedSharding, PartitionSpec

    from concourse import bass2jax

    nc = _build("full")
    bass2jax.install_neuronx_cc_hook()
    partition_name = nc.partition_id_tensor.name if nc.partition_id_tensor else None

    in_names, out_names, out_avals, zero_outs = [], [], [], []
    for alloc in nc.m.functions[0].allocations:
        if not isinstance(alloc, mybir.MemoryLocationSet):
            continue
        name = alloc.memorylocations[0].name
        if alloc.kind == "ExternalInput":
            if name != partition_name:
                in_names.append(name)
        elif alloc.kind == "ExternalOutput":
            shape = tuple(alloc.tensor_shape)
            dtype = mybir.dt.np(alloc.dtype)
            out_avals.append(jax.core.ShapedArray(shape, dtype))
            out_names.append(name)
            zero_outs.append(np.zeros(shape, dtype))

    n_params = len(in_names)
    all_in_names = list(in_names) + list(out_names)
    if partition_name is not None:
        all_in_names.append(partition_name)

    def _body(*args):
        operands = list(args)
        if partition_name is not None:
            operands.append(bass2jax.partition_id_tensor())
        return tuple(
            bass2jax._bass_exec_p.bind(
                *operands,
                out_avals=tuple(out_avals),
                in_names=tuple(all_in_names),
                out_names=tuple(out_names),
                lowering_input_output_aliases=(),
                sim_require_finite=True,
                sim_require_nnan=True,
                nc=nc,
            )
        )

    devices = jax.devices()[:N_CORES]
    mesh = Mesh(np.asarray(devices), ("core",))
    n_args = n_params + len(out_names)
    sharded = jax.jit(
        shard_map(
            _body,
            mesh=mesh,
            in_specs=(PartitionSpec("core"),) * n_args,
            out_specs=(PartitionSpec("core"),) * len(out_names),
            check_rep=False,
        ),
        keep_unused=True,
    )
    shard = NamedSharding(mesh, PartitionSpec("core"))
    zero_dev = [
        jax.device_put(
            np.zeros((N_CORES * z.shape[0], *z.shape[1:]), z.dtype), shard
        )
        for z in zero_outs
    ]
    cv_dev = jax.device_put(
        np.concatenate([_consts()] * N_CORES, axis=0), shard
    )

    def run(x_full: np.ndarray):
        args = []
        for n in in_names:
            if n == "x":
                args.append(jax.device_put(x_full, shard))
            elif n == "consts":
                args.append(cv_dev)
            else:
                raise KeyError(n)
        outs = sharded(*args, *zero_dev)
        return {
            n: np.asarray(outs[i]).reshape(N_CORES, *out_avals[i].shape)
            for i, n in enumerate(out_names)
        }

    return run


def kernel(inputs: np.ndarray, **_ignored):
    global _RUNNER_CACHE
    if _RUNNER_CACHE is None:
        _RUNNER_CACHE = _build_runner()

    inputs = np.ascontiguousarray(inputs, dtype=np.float32)
    res = _RUNNER_CACHE(inputs)
    downsampled = res["out"].reshape(B, HH, WW, C)
    max_norm_index = np.int32(res["idx"][0, 0, 0])
    return downsampled, max_norm_index
